# revision 1
# baseline (speedup 1.0000x reference)
"""Trainium2 Bass kernel for nn_ConvModule: LN -> 1x1 conv (D->2I) -> SwiGLU
-> depthwise conv (K=31) -> PReLU -> 1x1 conv (I->D).

Sharding: data-parallel over batch, 2 batches per core across 8 cores.

Per-core pipeline (all tokens N = 2*2048 = 4096):
  - LayerNorm in [token, D] layout (stats via ACT/GPSIMD accum), ln_g/ln_b
    folded into W1'/b1' so the normalize step is a single tensor_scalar.
  - PE-transpose xn tiles into [D, token] panels (fp32r) for GEMM1.
  - GEMM1 (fp32r, PSUM-accumulated over 4 D-chunks), SwiGLU epilogue on
    ACT/DVE/GPSIMD, result written as bf16 into padded per-channel-block
    time strips.
  - Depthwise conv = 31 PSUM-accumulated diagonal matmuls (bf16) per
    [128ch x 512t] output panel, with the tap shift realized as an SBUF
    read offset into the padded strip.
  - PReLU via one scalar_tensor_tensor: max(v, alpha*v) (alpha in [0,1)).
  - GEMM2 with v tiles as the stationary operand so the output lands in
    [token, D] layout; b2 added via a K=1 ones-row matmul into the same
    PSUM accumulation group.
"""

import sys

sys.path.insert(0, "/opt/trn_rl_repo")

from contextlib import ExitStack

import numpy as np

import concourse.bacc as bacc
import concourse.tile as tile
from concourse import mybir
from concourse.masks import make_identity
from concourse.bass_utils import run_bass_kernel_spmd

B, T, D, I, K = 16, 2048, 512, 1024, 31
NCORES = 8
BPC = B // NCORES  # batches per core
PAD = K // 2  # 15
E = 2 * I  # 2048
NTOK = BPC * T  # tokens per core
TP = T // 512  # time panels per batch (4)
ETILES = E // 128  # 16
CB = I // 128  # channel blocks (8)
DCH = D // 128  # d chunks (4)
STRIPW = PAD + T + PAD + 2  # 2080, 2-elem slack keeps width even
NPE = 26  # conv taps on the PE; taps [NPE, K) run on the vector engine

F32 = mybir.dt.float32
F32R = mybir.dt.float32r
BF16 = mybir.dt.bfloat16
ALU = mybir.AluOpType
ACTF = mybir.ActivationFunctionType
AX = mybir.AxisListType


def _build_kernel(ctx, tc):
    nc = tc.nc
    x_d = nc.dram_tensor("x", [BPC, T, D], F32, kind="ExternalInput").ap()
    ln_g_d = nc.dram_tensor("ln_g", [D], F32, kind="ExternalInput").ap()
    ln_b_d = nc.dram_tensor("ln_b", [D], F32, kind="ExternalInput").ap()
    w1_d = nc.dram_tensor("w1", [E, D], F32, kind="ExternalInput").ap()
    b1_d = nc.dram_tensor("b1", [E], F32, kind="ExternalInput").ap()
    dw_d = nc.dram_tensor("dw", [I, 1, K], F32, kind="ExternalInput").ap()
    dwb_d = nc.dram_tensor("dwb", [I], F32, kind="ExternalInput").ap()
    alpha_d = nc.dram_tensor("alpha", [I], F32, kind="ExternalInput").ap()
    w2_d = nc.dram_tensor("w2", [D, I], F32, kind="ExternalInput").ap()
    b2_d = nc.dram_tensor("b2", [D], F32, kind="ExternalInput").ap()
    out_d = nc.dram_tensor("out", [BPC, T, D], F32, kind="ExternalOutput").ap()

    P = 128

    const = ctx.enter_context(tc.tile_pool(name="const", bufs=1))
    psum = ctx.enter_context(tc.tile_pool(name="psum", bufs=8, space="PSUM"))

    ident = const.tile([P, P], F32, tag="ident")
    make_identity(nc, ident[:])
    ident_bf = const.tile([P, P], BF16, tag="ident_bf")
    make_identity(nc, ident_bf[:])

    # ---- small parameter tiles ----
    g_sb = const.tile([P, DCH], F32, tag="g_sb")
    nc.sync.dma_start(g_sb[:], ln_g_d.rearrange("(j p) -> p j", p=P))
    lnb_sb = const.tile([P, DCH], F32, tag="lnb_sb")
    nc.sync.dma_start(lnb_sb[:], ln_b_d.rearrange("(j p) -> p j", p=P))
    lnb_r = const.tile([P, DCH], F32R, tag="lnb_r")
    nc.vector.tensor_copy(lnb_r[:], lnb_sb[:])
    alpha_sb = const.tile([P, CB], F32, tag="alpha_sb")
    nc.sync.dma_start(alpha_sb[:], alpha_d.rearrange("(c p) -> p c", p=P))
    dwb_sb = const.tile([P, CB], F32, tag="dwb_sb")
    nc.sync.dma_start(dwb_sb[:], dwb_d.rearrange("(c p) -> p c", p=P))
    dw_sb = const.tile([P, CB * K], F32, tag="dw_sb")
    for cb in range(CB):
        nc.sync.dma_start(dw_sb[:, cb * K:(cb + 1) * K], dw_d[cb * P:(cb + 1) * P, 0, :])
    b2row_r = const.tile([1, D], F32R, tag="b2row_r")
    ones_r = const.tile([1, P], F32R, tag="ones_r")
    eps_t = const.tile([P, 1], F32, tag="eps_t")
    nc.vector.memset(eps_t[:], 1e-5)

    # ---- weight preprocessing: W1' = (w1 * ln_g)^T as fp32r [d, e] panels ----
    w1t = [const.tile([P, E], F32R, tag=f"w1t{j}", name=f"w1t{j}") for j in range(DCH)]
    w2t = [const.tile([P, D], BF16, tag=f"w2t{i}", name=f"w2t{i}") for i in range(CB)]
    b1p = const.tile([P, ETILES], F32, tag="b1p")
    b1scr_d = nc.dram_tensor("b1scr", [E], F32).ap()
    with tc.tile_pool(name="setup", bufs=2) as setup:
        b2row = setup.tile([1, D], F32, tag="b2row", bufs=1)
        nc.sync.dma_start(b2row[:], b2_d[None, :])
        nc.vector.tensor_copy(b2row_r[:], b2row[:])
        ones_f = setup.tile([1, P], F32, tag="ones_f", bufs=1)
        nc.vector.memset(ones_f[:], 1.0)
        nc.vector.tensor_copy(ones_r[:], ones_f[:])
        for i in range(ETILES):
            wnat = setup.tile([P, D], F32, tag="wnat", bufs=4)
            (nc.sync if i % 2 == 0 else nc.scalar).dma_start(
                wnat[:], w1_d[i * P:(i + 1) * P, :])
            for j in range(DCH):
                pt = psum.tile([P, P], F32, tag="ps")
                nc.tensor.transpose(pt[:], wnat[:, j * P:(j + 1) * P], ident[:])
                # scale rows (=d) by ln_g while copying out of PSUM
                nc.vector.tensor_scalar_mul(
                    w1t[j][:, i * P:(i + 1) * P], pt[:], g_sb[:, j:j + 1])
        # w2^T as bf16 [c, d] panels
        for jj in range(DCH):
            wnat2 = setup.tile([P, I], F32, tag="wnat2", bufs=2)
            nc.sync.dma_start(wnat2[:], w2_d[jj * P:(jj + 1) * P, :])
            for i in range(CB):
                pt2 = psum.tile([P, P], F32, tag="ps")
                nc.tensor.transpose(pt2[:], wnat2[:, i * P:(i + 1) * P], ident[:])
                nc.vector.tensor_copy(w2t[i][:, jj * P:(jj + 1) * P], pt2[:])

        # b1' = b1 + W1 @ ln_b. fp32r matmuls need a moving free dim >= 2,
        # so compute ln_b^T @ W1'^T as [1, 512] rows and bounce through DRAM
        # to the per-partition column layout.
        b1row = setup.tile([1, E], F32, tag="b1row", bufs=1)
        nc.sync.dma_start(b1row[:], b1_d[None, :])
        b1sum = setup.tile([1, E], F32, tag="b1sum", bufs=1)
        for jj in range(DCH):
            ps_r = psum.tile([1, 512], F32, tag="ps")
            for j in range(DCH):
                nc.tensor.matmul(
                    ps_r[:], lnb_r[:, j:j + 1], w1t[j][:, jj * 512:(jj + 1) * 512],
                    start=(j == 0), stop=(j == DCH - 1))
            nc.vector.tensor_add(
                b1sum[:, jj * 512:(jj + 1) * 512], ps_r[:],
                b1row[:, jj * 512:(jj + 1) * 512])
        nc.sync.dma_start(b1scr_d[None, :], b1sum[:])
        nc.sync.dma_start(b1p[:], b1scr_d.rearrange("(i p) -> p i", p=P))

    # ---- pools for the main loop ----
    xpool = ctx.enter_context(tc.tile_pool(name="xpool", bufs=2))
    xnpool = ctx.enter_context(tc.tile_pool(name="xnpool", bufs=5))
    stat = ctx.enter_context(tc.tile_pool(name="stat", bufs=12))
    scr = ctx.enter_context(tc.tile_pool(name="scr", bufs=2))
    xnt = ctx.enter_context(tc.tile_pool(name="xnt", bufs=6))
    sw = ctx.enter_context(tc.tile_pool(name="sw", bufs=2))
    strips = ctx.enter_context(tc.tile_pool(name="strips", bufs=8))
    diagp = ctx.enter_context(tc.tile_pool(name="diagp", bufs=3))
    vact = ctx.enter_context(tc.tile_pool(name="vact", bufs=32))
    wsbp = ctx.enter_context(tc.tile_pool(name="wsbp", bufs=2))
    outp = ctx.enter_context(tc.tile_pool(name="outp", bufs=2))

    def load_x_panel(b, tp):
        tiles = []
        for tt in range(4):
            t0 = tp * 512 + tt * P
            x_t = xpool.tile([P, D], F32, tag="x", bufs=10, name=f"x_{b}_{tp}_{tt}")
            nc.scalar.dma_start(x_t[:], x_d[b, t0:t0 + P, :])
            tiles.append(x_t)
        return tiles

    def emit_ln_panel(b, tp, x_tiles):
        """LayerNorm + PE-transpose for one 512-token panel -> xnT fp32r."""
        xn_tiles, means, negvs, stdvs = [], [], [], []
        for tt in range(4):
            x_t = x_tiles[tt]
            ssum = stat.tile([P, 1], F32, tag="ssum")
            ssq = stat.tile([P, 1], F32, tag="ssq")
            xcp = scr.tile([P, D], F32, tag="xsq")
            nc.scalar.activation(xcp[:], x_t[:], ACTF.Identity, accum_out=ssum[:])
            xsq = scr.tile([P, D], F32, tag="xsq")
            nc.scalar.activation(xsq[:], x_t[:], ACTF.Square, accum_out=ssq[:])
            mean = stat.tile([P, 1], F32, tag="mean")
            nc.vector.tensor_scalar_mul(mean[:], ssum[:], 1.0 / D)
            ex2 = stat.tile([P, 1], F32, tag="ex2")
            nc.vector.tensor_scalar_mul(ex2[:], ssq[:], 1.0 / D)
            negv = stat.tile([P, 1], F32, tag="negv")
            nc.vector.scalar_tensor_tensor(
                negv[:], mean[:], mean[:], ex2[:],
                op0=ALU.mult, op1=ALU.subtract)
            means.append(mean)
            negvs.append(negv)
        for tt in range(4):
            stdv = stat.tile([P, 1], F32, tag="stdv")
            nc.scalar.activation(stdv[:], negvs[tt][:], ACTF.Sqrt,
                                 scale=-1.0, bias=eps_t[:])
            stdvs.append(stdv)
        for tt in range(4):
            rstd = stat.tile([P, 1], F32, tag="rstd")
            nc.vector.reciprocal(rstd[:], stdvs[tt][:])
            xn_t = xnpool.tile([P, D], F32, tag="xn")
            nc.vector.tensor_scalar(
                xn_t[:], x_tiles[tt][:], means[tt][:], rstd[:],
                op0=ALU.subtract, op1=ALU.mult)
            xn_tiles.append(xn_t)
        xnt_p = []
        for j in range(DCH):
            ptr = psum.tile([P, 512], F32, tag="ps")
            for tt in range(4):
                nc.tensor.transpose(
                    ptr[:, tt * P:(tt + 1) * P],
                    xn_tiles[tt][:, j * P:(j + 1) * P], ident[:])
            xt = xnt.tile([P, 512], F32R, tag="xnt")
            nc.scalar.activation(xt[:], ptr[:], ACTF.Copy)
            xnt_p.append(xt)
        return xnt_p

    xq = {(0, 0): load_x_panel(0, 0)}
    xnt_cache = {(0, 0): emit_ln_panel(0, 0, xq.pop((0, 0)))}
    for b in range(BPC):
        # ---------- LN + GEMM1 + SwiGLU ----------
        strip = []
        for cb in range(CB):
            s = strips.tile([P, STRIPW], BF16, tag="strip")
            nc.gpsimd.memset(s[:, 0:PAD], 0.0)
            nc.gpsimd.memset(s[:, PAD + T:STRIPW], 0.0)
            strip.append(s)

        for tp in range(TP):
            if tp + 1 < TP:
                xq[(b, tp + 1)] = load_x_panel(b, tp + 1)
            elif b + 1 < BPC:
                xq[(b + 1, 0)] = load_x_panel(b + 1, 0)
            if (b, tp) in xnt_cache:
                xnt_p = xnt_cache.pop((b, tp))
            else:
                xnt_p = emit_ln_panel(b, tp, xq.pop((b, tp)))

            for i in range(CB):
                ps_a = psum.tile([P, 512], F32, tag="ps")
                ps_g = psum.tile([P, 512], F32, tag="ps")
                for j in range(DCH):
                    nc.tensor.matmul(
                        ps_a[:], w1t[j][:, i * P:(i + 1) * P], xnt_p[j][:],
                        start=(j == 0), stop=(j == DCH - 1))
                for j in range(DCH):
                    ii = i + CB
                    nc.tensor.matmul(
                        ps_g[:], w1t[j][:, ii * P:(ii + 1) * P], xnt_p[j][:],
                        start=(j == 0), stop=(j == DCH - 1))
                # u = (a + b1a) * silu(g + b1g), silu fused as two stt ops
                s_sb = sw.tile([P, 512], F32, tag="s_sb")
                nc.scalar.activation(
                    s_sb[:], ps_g[:], ACTF.Sigmoid, bias=b1p[:, i + CB:i + CB + 1])
                t1_sb = sw.tile([P, 512], F32, tag="t1_sb")
                nc.vector.scalar_tensor_tensor(
                    t1_sb[:], ps_g[:], b1p[:, i + CB:i + CB + 1], s_sb[:],
                    op0=ALU.add, op1=ALU.mult)
                nc.vector.scalar_tensor_tensor(
                    strip[i][:, PAD + tp * 512:PAD + (tp + 1) * 512],
                    ps_a[:], b1p[:, i:i + 1], t1_sb[:],
                    op0=ALU.add, op1=ALU.mult)

        # ---------- depthwise conv + PReLU ----------
        vpan = [[None] * TP for _ in range(CB)]
        diags = {}

        def build_diag(cb):
            dg = diagp.tile([P, K * P], BF16, tag="diag", name=f"dg_{b}_{cb}")
            for tap in range(K):
                nc.vector.tensor_scalar_mul(
                    dg[:, tap * P:(tap + 1) * P], ident_bf[:],
                    dw_sb[:, cb * K + tap:cb * K + tap + 1])
            diags[cb] = dg

        build_diag(0)
        for cb in range(CB):
            if cb + 1 < CB:
                build_diag(cb + 1)
            dg = diags.pop(cb)
            for tp in range(TP):
                ps_c = psum.tile([P, 512], F32, tag="ps")
                for tap in range(NPE):
                    off = tp * 512 + tap
                    nc.tensor.matmul(
                        ps_c[:], dg[:, tap * P:(tap + 1) * P],
                        strip[cb][:, off:off + 512],
                        start=(tap == 0), stop=(tap == NPE - 1))
                # remaining taps on DVE; chain seeded with the dwb bias
                td = wsbp.tile([P, 512], F32, tag="td")
                nc.vector.tensor_scalar(
                    td[:], strip[cb][:, tp * 512 + NPE:tp * 512 + NPE + 512],
                    dw_sb[:, cb * K + NPE:cb * K + NPE + 1],
                    dwb_sb[:, cb:cb + 1], op0=ALU.mult, op1=ALU.add)
                for tap in range(NPE + 1, K):
                    nc.vector.scalar_tensor_tensor(
                        td[:], strip[cb][:, tp * 512 + tap:tp * 512 + tap + 512],
                        dw_sb[:, cb * K + tap:cb * K + tap + 1], td[:],
                        op0=ALU.mult, op1=ALU.add)
                w_sb = wsbp.tile([P, 512], F32, tag="w_sb")
                nc.vector.tensor_add(w_sb[:], ps_c[:], td[:])
                vt = vact.tile([P, 512], BF16, tag="vact")
                nc.vector.scalar_tensor_tensor(
                    vt[:], w_sb[:], alpha_sb[:, cb:cb + 1], w_sb[:],
                    op0=ALU.mult, op1=ALU.max)
                vpan[cb][tp] = vt

        # ---------- GEMM2 ----------
        for tp in range(TP):
            for tt in range(4):
                ps_o = psum.tile([P, D], F32, tag="ps")
                nc.tensor.matmul(ps_o[:], ones_r[:], b2row_r[:],
                                 start=True, stop=False)
                for cb in range(CB):
                    nc.tensor.matmul(
                        ps_o[:], vpan[cb][tp][:, tt * P:(tt + 1) * P], w2t[cb][:],
                        start=False, stop=(cb == CB - 1))
                o_sb = outp.tile([P, D], F32, tag="o_sb")
                nc.scalar.activation(o_sb[:], ps_o[:], ACTF.Copy)
                t0 = tp * 512 + tt * P
                nc.sync.dma_start(out_d[b, t0:t0 + P, :], o_sb[:])


_NC_CACHE = None


def _get_program():
    global _NC_CACHE
    if _NC_CACHE is None:
        nc = bacc.Bacc("TRN2", target_bir_lowering=False, debug=False)
        with tile.TileContext(nc) as tc, ExitStack() as ctx:
            _build_kernel(ctx, tc)
        nc.compile()
        _NC_CACHE = nc
    return _NC_CACHE


def kernel(x, ln_g, ln_b, w1, b1, dw, dwb, alpha, w2, b2, _trace=False):
    nc = _get_program()
    x = np.ascontiguousarray(x, dtype=np.float32)
    shared = {
        "ln_g": np.ascontiguousarray(ln_g, np.float32),
        "ln_b": np.ascontiguousarray(ln_b, np.float32),
        "w1": np.ascontiguousarray(w1, np.float32),
        "b1": np.ascontiguousarray(b1, np.float32),
        "dw": np.ascontiguousarray(dw, np.float32),
        "dwb": np.ascontiguousarray(dwb, np.float32),
        "alpha": np.ascontiguousarray(alpha, np.float32),
        "w2": np.ascontiguousarray(w2, np.float32),
        "b2": np.ascontiguousarray(b2, np.float32),
    }
    in_maps = [
        {"x": x[c * BPC:(c + 1) * BPC], **shared} for c in range(NCORES)
    ]
    res = run_bass_kernel_spmd(nc, in_maps, core_ids=list(range(NCORES)),
                               trace=_trace)
    out = np.concatenate([res.results[c]["out"] for c in range(NCORES)], axis=0)
    if _trace:
        kernel.last_results = res
    return out

